# revision 16
# baseline (speedup 1.0000x reference)
"""EntityAwareAttention TRN2 Bass kernel — 8-core data parallel.

Problem (per full batch): B=64, L=256, H=1024, P=64, A=512, T=8.
  e1_h/e2_h   = word_hiddens gathered at e1_end/e2_end           [B, H]
  e*_type     = softmax(e_h @ tE.T) @ tE                          [B, H]
  ef          = concat(e1_h, e1_type, e2_h, e2_type)              [B, 4H]
  dense_pos   = concat(wh, pos_e1, pos_e2) @ W_pos                [B, L, A]
  dense_ent   = ef @ W_ent                                        [B, A]
  u           = tanh(dense_pos + repeat-interleave(dense_ent))    [B, L, A]
                (addend for (l, a) is dense_ent[b, 2l + (a>=256)])
  vu          = u @ v ; alpha = softmax(vu, axis=L)               [B, L]
  z           = sum_l alpha[b,l] * wh[b,l,:]                      [B, H]

Sharding: batch across 8 cores (8 batches/core); weights replicated.

Per-core layout strategy:
  * tokens t = b*L + l, 16 token-tiles of 128.
  * Big matmul runs as out[t_tile, A] += whT_k.T @ W_pos_k over 9 k-tiles
    of the f=H+2P contraction dim; activations are PE-transposed on the fly
    ([128,128] f32 transposes; rounding to fp32r happens in the PSUM->SBUF
    staging copy). fp32r matmuls run at full PE rate for N>=256.
  * dense_ent addend is applied as a per-partition bias in the ACT tanh
    (two halves a<256 / a>=256), using a transposed, parity-split copy of
    dense_ent.
  * vu = one fused DVE tensor_tensor_reduce per token-tile.
  * softmax over L via tiny PE transposes to an [8, 256] batch-major view.
  * z computed transposed ([H-chunk, b] PSUM accumulation), transposed back.
"""

import numpy as np

import concourse.bass as bass
import concourse.tile as tile
from concourse import bacc, mybir
from concourse.bass_utils import run_bass_kernel_spmd

F32 = mybir.dt.float32
F32R = mybir.dt.float32r
I32 = mybir.dt.int32
AF = mybir.ActivationFunctionType
ALU = mybir.AluOpType

B, L, H, P2, A, T = 64, 256, 1024, 64, 512, 8
NCORES = 8
BL = B // NCORES            # 8 local batches
TOK = BL * L                # 2048 tokens
NT = TOK // 128             # 16 token tiles
F = H + 2 * P2              # 1152 contraction dim
KF = F // 128               # 9 k-tiles
KE = 4 * H // 128           # 32 W_ent k-tiles
HC = H // 128               # 8 h-chunks


def _build_core(tc, stage=99):
    nc = tc.nc
    wh_d = nc.dram_tensor("word_hiddens", [TOK, H], F32, kind="ExternalInput").ap()
    pe1_d = nc.dram_tensor("pos_e1", [TOK, P2], F32, kind="ExternalInput").ap()
    pe2_d = nc.dram_tensor("pos_e2", [TOK, P2], F32, kind="ExternalInput").ap()
    e1_d = nc.dram_tensor("e1_end", [BL, 1], I32, kind="ExternalInput").ap()
    e2_d = nc.dram_tensor("e2_end", [BL, 1], I32, kind="ExternalInput").ap()
    te_d = nc.dram_tensor("type_embeddings", [T, H], F32, kind="ExternalInput").ap()
    wpos_d = nc.dram_tensor("W_pos", [F, A], F32, kind="ExternalInput").ap()
    went_d = nc.dram_tensor("W_ent", [4 * H, A], F32, kind="ExternalInput").ap()
    v_d = nc.dram_tensor("v", [1, A], F32, kind="ExternalInput").ap()
    out_d = nc.dram_tensor("out", [BL, H], F32, kind="ExternalOutput").ap()

    const = tc.alloc_tile_pool(name="const", bufs=1)
    wh_pool = tc.alloc_tile_pool(name="wh", bufs=NT)
    stage_p = tc.alloc_tile_pool(name="stage", bufs=6)
    work = tc.alloc_tile_pool(name="work", bufs=2)
    went_pool = tc.alloc_tile_pool(name="went", bufs=4)
    ps_dp = tc.alloc_tile_pool(name="ps_dp", bufs=3, space="PSUM")
    ps_tr = tc.alloc_tile_pool(name="ps_tr", bufs=2, space="PSUM")
    ps_sm = tc.alloc_tile_pool(name="ps_sm", bufs=3, space="PSUM")

    import os
    act_f32r_ok = os.environ.get("ACT_F32R", "1") == "1"

    def copy(eng, out, in_):
        if eng is nc.scalar:
            if act_f32r_ok or out.dtype != F32R:
                nc.scalar.copy(out=out, in_=in_)
            else:
                nc.vector.tensor_copy(out=out, in_=in_)
        else:
            eng.tensor_copy(out=out, in_=in_)

    # ---- constants / small loads ----
    iota_p = const.tile([128, 128], I32)
    iota_f = const.tile([128, 128], I32)
    nc.gpsimd.iota(iota_p[:], pattern=[[0, 128]], base=0, channel_multiplier=1)
    nc.gpsimd.iota(iota_f[:], pattern=[[1, 128]], base=0, channel_multiplier=0)
    ident = const.tile([128, 128], F32)
    nc.vector.tensor_tensor(out=ident[:], in0=iota_p[:], in1=iota_f[:],
                            op=ALU.is_equal)

    v_sb = const.tile([1, A], F32)
    nc.sync.dma_start(v_sb[:], v_d[:])
    v_bc = const.tile([128, A], F32)
    nc.gpsimd.partition_broadcast(v_bc[:], v_sb[0:1, :])

    te_sb = const.tile([T, H], F32)
    nc.sync.dma_start(te_sb[:], te_d[:])
    te_r = const.tile([T, H], F32R)
    nc.vector.tensor_copy(out=te_r[:], in_=te_sb[:])

    # gather indices: flat token index b*L + e_end[b], for e1 rows 0..7 and
    # e2 rows 8..15
    ends = const.tile([2 * BL, 1], I32)
    nc.sync.dma_start(ends[0:BL, :], e1_d[:])
    nc.sync.dma_start(ends[BL:2 * BL, :], e2_d[:])
    gidx = const.tile([2 * BL, 1], I32)
    nc.gpsimd.iota(gidx[:], pattern=[[1, 1]], base=0, channel_multiplier=L)
    # p*L mod (BL*L): BL*L is a power of two, so a mask folds e2 rows back
    nc.vector.tensor_scalar(out=gidx[:], in0=gidx[:],
                            scalar1=BL * L - 1, scalar2=None,
                            op0=ALU.bitwise_and)
    nc.vector.tensor_tensor(out=gidx[:], in0=gidx[:], in1=ends[:], op=ALU.add)

    eh = const.tile([2 * BL, H], F32)
    nc.gpsimd.indirect_dma_start(
        out=eh[:], out_offset=None, in_=wh_d[:],
        in_offset=bass.IndirectOffsetOnAxis(ap=gidx[:, 0:1], axis=0))

    if stage <= 0:
        # debug: dump gathered rows
        nc.sync.dma_start(out_d[:], eh[0:BL, :])
        for p in (ps_sm, ps_tr, ps_dp, went_pool, work, stage_p, wh_pool, const):
            p.release()
        return

    # ---- W_pos load + fp32r round ----
    wpos_f = const.tile([128, KF * A], F32)
    for k in range(KF):
        nc.sync.dma_start(wpos_f[:, k * A:(k + 1) * A],
                          wpos_d[k * 128:(k + 1) * 128, :])
    wpos = const.tile([128, KF * A], F32R)
    for k in range(KF):
        eng = nc.vector if k % 2 == 0 else nc.scalar
        copy(eng, wpos[:, k * A:(k + 1) * A], wpos_f[:, k * A:(k + 1) * A])

    # ---- wh loads: first two tiles early so PE can start ----
    wh_sb = []
    for i in range(NT):
        wh_sb.append(wh_pool.tile([128, H], F32, tag="wh", name=f"wh{i}"))
    for i in range(2):
        nc.sync.dma_start(wh_sb[i][:], wh_d[i * 128:(i + 1) * 128, :])

    # ---- entity feature transposes:  efT[:, kt*8:+8] over 32 k-tiles ----
    # region 0: e1_h (kt 0..7), 1: e1_type (8..15), 2: e2_h (16..23),
    # 3: e2_type (24..31)
    efT = const.tile([128, KE * BL], F32R)
    for hc in range(HC):
        pt = ps_tr.tile([128, 128], F32, tag="tr")
        nc.tensor.transpose(pt[:, 0:2 * BL], eh[:, hc * 128:(hc + 1) * 128],
                            ident[0:2 * BL, 0:2 * BL])
        nc.vector.tensor_copy(out=efT[:, hc * BL:(hc + 1) * BL],
                              in_=pt[:, 0:BL])
        nc.scalar.copy(out=efT[:, (2 * HC + hc) * BL:(2 * HC + hc + 1) * BL],
                      in_=pt[:, BL:2 * BL])

    # tE^T  [H(p-tiles), T] as [128, HC*T], fp32r
    teT = const.tile([128, HC * T], F32R)
    for hc in range(HC):
        pt = ps_tr.tile([128, 128], F32, tag="tr")
        nc.tensor.transpose(pt[:, 0:T], te_sb[:, hc * 128:(hc + 1) * 128],
                            ident[0:T, 0:T])
        nc.vector.tensor_copy(out=teT[:, hc * T:(hc + 1) * T], in_=pt[:, 0:T])

    # scores + softmax + e_type^T for each entity
    for ent in range(2):
        sc = ps_sm.tile([BL, T], F32, tag="sm")
        for hc in range(HC):
            col = (0 if ent == 0 else 2 * HC) + hc
            nc.tensor.matmul(sc[:], lhsT=efT[:, col * BL:(col + 1) * BL],
                             rhs=teT[:, hc * T:(hc + 1) * T],
                             start=(hc == 0), stop=(hc == HC - 1))
        asm = const.tile([BL, T], F32, tag=f"asm{ent}")
        ssum = const.tile([BL, 1], F32, tag=f"ssum{ent}")
        nc.scalar.activation(asm[:], sc[:], AF.Exp, accum_out=ssum[:])
        rs = const.tile([BL, 1], F32, tag=f"rs{ent}")
        nc.vector.reciprocal(rs[:], ssum[:])
        al = const.tile([BL, T], F32, tag=f"al{ent}")
        nc.vector.tensor_scalar(out=al[:], in0=asm[:], scalar1=rs[:, 0:1],
                                scalar2=None, op0=ALU.mult)
        pt = ps_tr.tile([128, 128], F32, tag="tr")
        nc.tensor.transpose(pt[0:T, 0:BL], al[:], ident[0:BL, 0:BL])
        alT = const.tile([T, BL], F32R, tag=f"alT{ent}")
        nc.vector.tensor_copy(out=alT[:], in_=pt[0:T, 0:BL])
        # e_type^T chunks into efT region 1 / 3
        for hc in range(HC):
            pe = ps_sm.tile([128, BL], F32, tag="sm")
            nc.tensor.matmul(pe[:], lhsT=te_r[:, hc * 128:(hc + 1) * 128],
                             rhs=alT[:], start=True, stop=True)
            col = (HC if ent == 0 else 3 * HC) + hc
            eng = nc.vector if hc % 2 == 0 else nc.scalar
            copy(eng, efT[:, col * BL:(col + 1) * BL], pe[:])

    if stage <= 1:
        nc.vector.tensor_copy(out=eh[0:BL, 0:KE * BL].bitcast(F32), in_=efT[:, :].rearrange("p c -> c p")[0:BL, :].bitcast(F32)) if False else None
        # debug: dump efT columns (transposed back on DMA is awkward; just dump raw)
        nc.sync.dma_start(out_d[:, 0:KE * BL], efT[0:BL, :].bitcast(F32))
        nc.sync.dma_start(out_d[:, KE * BL:2 * KE * BL], efT[BL:2 * BL, :].bitcast(F32))
        for p in (ps_sm, ps_tr, ps_dp, went_pool, work, stage_p, wh_pool, const):
            p.release()
        return

    # ---- W_ent stream (fp32r cast on SWDGE dma) + dense_ent ----
    de = ps_sm.tile([BL, A], F32, tag="sm")
    went_tiles = []
    for k in range(KE):
        wt = went_pool.tile([128, A], F32R, tag="went", name=f"went{k}")
        nc.gpsimd.dma_start(wt[:], went_d[k * 128:(k + 1) * 128, :])
        went_tiles.append(wt)
    for k in range(KE):
        nc.tensor.matmul(de[:], lhsT=efT[:, k * BL:(k + 1) * BL],
                         rhs=went_tiles[k][:], start=(k == 0), stop=(k == KE - 1))

    # parity split (even a's then odd a's) + transpose to [l(p), b] bias cols
    de_eo = const.tile([BL, A], F32)
    nc.vector.tensor_copy(
        out=de_eo.rearrange("b (two l) -> b two l", two=2),
        in_=de.rearrange("b (l two) -> b two l", two=2))
    # bias_sb cols: parity*16 + half*8 + b
    bias_sb = const.tile([128, 32], F32)
    for par in range(2):
        for half in range(2):
            pt = ps_tr.tile([128, 128], F32, tag="tr")
            src = de_eo[:, par * 256 + half * 128: par * 256 + (half + 1) * 128]
            nc.tensor.transpose(pt[:, 0:BL], src, ident[0:BL, 0:BL])
            nc.vector.tensor_copy(
                out=bias_sb[:, par * 16 + half * 8: par * 16 + half * 8 + BL],
                in_=pt[:, 0:BL])

    if stage <= 2:
        nc.sync.dma_start(out_d[0:BL, 0:32], bias_sb[0:BL, :])
        for p in (ps_sm, ps_tr, ps_dp, went_pool, work, stage_p, wh_pool, const):
            p.release()
        return

    # ---- pos embeddings: strided load so tile i is [:, i*128:(i+1)*128]
    # with e1 in cols 0:64, e2 in 64:128 of each block ----
    pos_sb = const.tile([128, NT * 128], F32)
    nc.sync.dma_start(
        pos_sb.rearrange("p (i c) -> p i c", c=128)[:, :, 0:P2],
        pe1_d.rearrange("(i p) c -> p i c", p=128))
    nc.sync.dma_start(
        pos_sb.rearrange("p (i c) -> p i c", c=128)[:, :, P2:128],
        pe2_d.rearrange("(i p) c -> p i c", p=128))

    # ---- remaining wh loads ----
    for i in range(2, NT):
        nc.sync.dma_start(wh_sb[i][:], wh_d[i * 128:(i + 1) * 128, :])

    # ---- main loop over token tiles ----
    vu0 = const.tile([128, BL], F32)   # vu for l in [0,128), col = b
    vu1 = const.tile([128, BL], F32)   # vu for l in [128,256), col = b
    for i in range(NT):
        b, half = i // 2, i % 2
        dp = ps_dp.tile([128, A], F32, tag="dp")
        for k in range(KF):
            if k < HC:
                src = wh_sb[i][:, k * 128:(k + 1) * 128]
            else:
                src = pos_sb[:, i * 128:(i + 1) * 128]
            st = stage_p.tile([128, 128], F32R, tag="stage")
            pt = ps_tr.tile([128, 128], F32, tag="tr")
            nc.tensor.transpose(pt[:], src, ident[:])
            eng = nc.vector if k % 2 == 0 else nc.scalar
            copy(eng, st[:], pt[:])
            nc.tensor.matmul(dp[:], lhsT=st[:], rhs=wpos[:, k * A:(k + 1) * A],
                             start=(k == 0), stop=(k == KF - 1))
        u = work.tile([128, A], F32, tag="u")
        sub = int(os.environ.get("SUBSTAGE", "9"))
        vu_dst = (vu0 if half == 0 else vu1)
        if sub <= 0:
            nc.vector.tensor_copy(out=u[:], in_=dp[:])
            nc.vector.tensor_copy(out=vu_dst[:, b:b + 1], in_=u[:, 0:1])
            continue
        nc.scalar.activation(u[:, 0:256], dp[:, 0:256], AF.Tanh,
                             bias=bias_sb[:, half * 8 + b: half * 8 + b + 1])
        nc.scalar.activation(u[:, 256:512], dp[:, 256:512], AF.Tanh,
                             bias=bias_sb[:, 16 + half * 8 + b: 16 + half * 8 + b + 1])
        if sub <= 1:
            nc.vector.tensor_copy(out=vu_dst[:, b:b + 1], in_=u[:, 0:1])
            continue
        scr = work.tile([128, A], F32, tag="scr")
        if os.environ.get("TTR", "1") == "1":
            nc.vector.tensor_tensor_reduce(
                out=scr[:], in0=u[:], in1=v_bc[:], scale=1.0, scalar=0.0,
                op0=ALU.mult, op1=ALU.add, accum_out=vu_dst[:, b:b + 1])
        else:
            nc.vector.tensor_tensor(out=scr[:], in0=u[:], in1=v_bc[:],
                                    op=ALU.mult)
            nc.vector.tensor_reduce(out=vu_dst[:, b:b + 1], in_=scr[:],
                                    axis=mybir.AxisListType.X, op=ALU.add)

    if stage <= 3:
        nc.sync.dma_start(out_d[0:BL, 0:BL], vu0[0:BL, :])
        for p in (ps_sm, ps_tr, ps_dp, went_pool, work, stage_p, wh_pool, const):
            p.release()
        return

    # ---- softmax over L ----
    vubl = const.tile([BL, L], F32)
    for half, vt in ((0, vu0), (1, vu1)):
        pt = ps_tr.tile([128, 128], F32, tag="tr")
        nc.tensor.transpose(pt[0:BL, :], vt[:], ident[:])
        nc.vector.tensor_copy(out=vubl[:, half * 128:(half + 1) * 128],
                              in_=pt[0:BL, :])
    expv = const.tile([BL, L], F32)
    esum = const.tile([BL, 1], F32)
    nc.scalar.activation(expv[:], vubl[:], AF.Exp, accum_out=esum[:])
    ers = const.tile([BL, 1], F32)
    nc.vector.reciprocal(ers[:], esum[:])
    albl = const.tile([BL, L], F32)
    nc.vector.tensor_scalar(out=albl[:], in0=expv[:],
                            scalar1=ers[:, 0:1], scalar2=None, op0=ALU.mult)
    alT = const.tile([128, 2 * BL], F32)  # cols: half*8 + b
    for half in range(2):
        pt = ps_tr.tile([128, 128], F32, tag="tr")
        nc.tensor.transpose(pt[:, 0:BL], albl[:, half * 128:(half + 1) * 128],
                            ident[0:BL, 0:BL])
        nc.vector.tensor_copy(out=alT[:, half * BL:(half + 1) * BL],
                              in_=pt[:, 0:BL])

    # ---- z = alpha-weighted sum of wh rows, computed transposed ----
    z_sb = const.tile([BL, H], F32)
    for hc in range(HC):
        zt = ps_dp.tile([128, BL], F32, tag="dp")
        for b in range(BL):
            for half in range(2):
                i = b * 2 + half
                nc.tensor.matmul(
                    zt[:, b:b + 1],
                    lhsT=wh_sb[i][:, hc * 128:(hc + 1) * 128],
                    rhs=alT[:, half * BL + b: half * BL + b + 1],
                    start=(b == 0 and half == 0),
                    stop=(b == BL - 1 and half == 1))
        zts = work.tile([128, BL], F32, tag="zts")
        nc.vector.tensor_copy(out=zts[:], in_=zt[:])
        pt = ps_tr.tile([128, 128], F32, tag="tr")
        nc.tensor.transpose(pt[0:BL, :], zts[:], ident[:])
        eng = nc.vector if hc % 2 == 0 else nc.scalar
        copy(eng, z_sb[:, hc * 128:(hc + 1) * 128], pt[0:BL, :])

    nc.sync.dma_start(out_d[:], z_sb[:])

    for p in (ps_sm, ps_tr, ps_dp, went_pool, work, stage_p, wh_pool, const):
        p.release()


def build(stage=99):
    nc = bacc.Bacc("TRN2", target_bir_lowering=False, debug=False,
                   num_devices=NCORES)
    with tile.TileContext(nc) as tc:
        _build_core(tc, stage)
    nc.compile()
    return nc


_NC = None


def kernel(word_hiddens, pos_e1_embeddings, pos_e2_embeddings, e1_end, e2_end,
           type_embeddings, W_pos, W_ent, v):
    global _NC
    if _NC is None:
        _NC = build()
    wh = np.ascontiguousarray(word_hiddens, dtype=np.float32).reshape(B, L, H)
    p1 = np.ascontiguousarray(pos_e1_embeddings, dtype=np.float32).reshape(B, L, P2)
    p2 = np.ascontiguousarray(pos_e2_embeddings, dtype=np.float32).reshape(B, L, P2)
    e1 = np.asarray(e1_end, dtype=np.int32).reshape(B)
    e2 = np.asarray(e2_end, dtype=np.int32).reshape(B)
    te = np.ascontiguousarray(type_embeddings, dtype=np.float32)
    wp = np.ascontiguousarray(W_pos, dtype=np.float32)
    we = np.ascontiguousarray(W_ent, dtype=np.float32)
    vv = np.ascontiguousarray(v, dtype=np.float32).reshape(1, A)

    in_maps = []
    for c in range(NCORES):
        s = slice(c * BL, (c + 1) * BL)
        in_maps.append({
            "word_hiddens": wh[s].reshape(TOK, H),
            "pos_e1": p1[s].reshape(TOK, P2),
            "pos_e2": p2[s].reshape(TOK, P2),
            "e1_end": e1[s].reshape(BL, 1),
            "e2_end": e2[s].reshape(BL, 1),
            "type_embeddings": te,
            "W_pos": wp,
            "W_ent": we,
            "v": vv,
        })
    res = run_bass_kernel_spmd(_NC, in_maps, core_ids=list(range(NCORES)))
    return np.concatenate([res.results[c]["out"] for c in range(NCORES)], axis=0)
